# revision 1
# baseline (speedup 1.0000x reference)
"""ButterflyLinear TRN2 kernel — 8-core data-parallel dense matmul.

The module  out = blockdiag(shuffle(blockdiag(x, stage1)) @ mix_w.T, stage2)
is a fixed linear map on the 2048-d feature axis.  We fold
stage1 ∘ shuffle ∘ mix ∘ stage2 into a single dense A [2048, 2048] on the
host (cheap block-wise einsums, fp64), then each NeuronCore computes
yT = A.T @ xT for its 2048-token shard: feature-major layout so the
contraction dim sits on SBUF partitions.  Operands are fp16 on the device
(FWL-fast weight loads, half the DMA bytes); accumulation is fp32 PSUM and
the output is fp32.  End-to-end relative error ~7e-4.
"""

import sys

if "/opt/trn_rl_repo" not in sys.path:
    sys.path.insert(0, "/opt/trn_rl_repo")

import numpy as np

IN_F = 2048
OUT_F = 2048
BS = 64
NIB = IN_F // BS
NOB = OUT_F // BS
N_CORES = 8
TOK_PC = 2048  # tokens per core (16384 / 8)

P = 128
KT = IN_F // P  # 16 k-tiles
MT = OUT_F // P  # 16 m-tiles
NT = 512  # token tile (matmul moving dim)
NN = TOK_PC // NT  # 4 token tiles per core

_CACHE = {}


def _build(repeats: int = 1, loop_iters: int = 0):
    """Build + compile the per-core Bass program (SPMD, same on all cores).

    loop_iters > 0 wraps the body in a hardware For_i loop (timing builds)."""
    import contextlib

    import concourse.mybir as mybir
    import concourse.tile as tile
    from concourse import bacc

    nc = bacc.Bacc(None, target_bir_lowering=False, debug=False)
    f32 = mybir.dt.float32
    f16 = mybir.dt.float16

    # A is host-pretiled to [m, p, k, c] so each per-m load is 4KB-contiguous
    # per partition.  x is [in_feat, tokens] (feature-major).
    a_ext = nc.declare_dram_parameter("a", [MT, P, KT, P], f16, isOutput=False)
    x_ext = nc.declare_dram_parameter("x", [IN_F, TOK_PC], f16, isOutput=False)
    y_ext = nc.declare_dram_parameter("y", [OUT_F, TOK_PC], f32, isOutput=True)

    x_v = x_ext[:].rearrange("(k p) t -> p k t", p=P)

    with tile.TileContext(nc) as tc:
        with (
            tc.tile_pool(name="a_pool", bufs=1) as a_pool,
            tc.tile_pool(name="x_pool", bufs=3) as x_pool,
            tc.tile_pool(name="o_pool", bufs=6) as o_pool,
            tc.tile_pool(name="ps", bufs=6, space="PSUM") as ps_pool,
        ):
            loop_cm = (
                tc.For_i(0, loop_iters, 1, hint_engines=(mybir.EngineType.PE,))
                if loop_iters
                else contextlib.nullcontext()
            )
            with loop_cm:
                for _rep in range(repeats):
                    # A fully resident, loaded in m-columns so compute can
                    # start as soon as the first column lands.  Scalar-engine
                    # HWDGE ring so A loads don't head-of-line block the
                    # X-slab loads issued on the sync ring.
                    a_t = []
                    for m in range(MT):
                        at = a_pool.tile([P, KT, P], f16, tag=f"a{m}")
                        nc.scalar.dma_start(at[:], a_ext[m])
                        a_t.append(at)
                    for n in range(NN):
                        x_t = x_pool.tile([P, KT, NT], f16, tag="x")
                        for k in range(KT):
                            nc.sync.dma_start(
                                x_t[:, k, :], x_v[:, k, n * NT : (n + 1) * NT]
                            )
                        for m in range(MT):
                            ps = ps_pool.tile([P, NT], f32)
                            for k in range(KT):
                                nc.tensor.matmul(
                                    ps[:],
                                    a_t[m][:, k, :],
                                    x_t[:, k, :],
                                    start=(k == 0),
                                    stop=(k == KT - 1),
                                )
                            o_t = o_pool.tile([P, NT], f32, tag="o")
                            nc.vector.tensor_copy(o_t[:], ps[:])
                            # gpsimd SWDGE ring: output stores must not
                            # share the sync FIFO with X-slab loads
                            nc.gpsimd.dma_start(
                                y_ext[m * P : (m + 1) * P, n * NT : (n + 1) * NT],
                                o_t[:],
                            )
    nc.compile()
    return nc


def get_nc(repeats: int = 1, loop_iters: int = 0):
    key = ("nc", repeats, loop_iters)
    if key not in _CACHE:
        _CACHE[key] = _build(repeats, loop_iters)
    return _CACHE[key]


def compose_A(stage1: np.ndarray, stage2: np.ndarray, mix_w: np.ndarray) -> np.ndarray:
    """Fold stage1 ∘ shuffle ∘ mix ∘ stage2 into one dense [IN_F, OUT_F] map."""
    j = np.arange(IN_F)
    sig = (j % BS) * NIB + j // BS  # shuffle: h2[:, j] = h1[:, sig(j)]
    Ms = np.empty((IN_F, OUT_F), dtype=np.float64)
    Ms[sig, :] = mix_w.T.astype(np.float64)  # y = h1 @ Ms
    A_mid = np.einsum(
        "gcd,gdo->gco",
        stage1.reshape(NIB, BS, BS).astype(np.float64),
        Ms.reshape(NIB, BS, OUT_F),
    ).reshape(IN_F, OUT_F)
    A = np.einsum(
        "igc,gcd->igd",
        A_mid.reshape(IN_F, NOB, BS),
        stage2.reshape(NOB, BS, BS).astype(np.float64),
    ).reshape(IN_F, OUT_F)
    return A.astype(np.float32)


def tile_A(A: np.ndarray) -> np.ndarray:
    """[IN_F, OUT_F] fp32 -> [m, p, k, c] fp16 device layout."""
    return np.ascontiguousarray(
        A.reshape(KT, P, MT, P).transpose(2, 1, 0, 3).astype(np.float16)
    )


def make_in_maps(x, stage1, stage2, mix_w):
    A = compose_A(np.asarray(stage1), np.asarray(stage2), np.asarray(mix_w))
    A_dev = tile_A(A)
    x_flat = np.ascontiguousarray(np.asarray(x), dtype=np.float32).reshape(-1, IN_F)
    in_maps = []
    for c in range(N_CORES):
        shard = x_flat[c * TOK_PC : (c + 1) * TOK_PC, :]
        xT = np.ascontiguousarray(shard.T.astype(np.float16))
        in_maps.append({"a": A_dev, "x": xT})
    return in_maps


def assemble_output(results, batch_shape):
    y_flat = np.empty((N_CORES * TOK_PC, OUT_F), dtype=np.float32)
    for c in range(N_CORES):
        y_flat[c * TOK_PC : (c + 1) * TOK_PC, :] = results[c]["y"].T
    return y_flat.reshape(*batch_shape, OUT_F)


def kernel(x, stage1, stage2, mix_w):
    from concourse.bass_utils import run_bass_kernel_spmd

    batch_shape = np.asarray(x).shape[:-1]
    nc = get_nc()
    in_maps = make_in_maps(x, stage1, stage2, mix_w)
    res = run_bass_kernel_spmd(nc, in_maps, list(range(N_CORES)))
    return assemble_output(res.results, batch_shape)

